# revision 18
# baseline (speedup 1.0000x reference)
"""GCN critic network kernel for 8 TRN2 NeuronCores.

Strategy (dst-shard graph parallel, fp8 pair-row gather):
  - Host packs dst nodes into 8x50 degree-balanced bins of 128 slots
    (46 ranks at <=2048 edges + 4 drained ranks at <=1536) so every core
    runs a uniform 784-chunk schedule with ~0.35% gather padding;
    self-loops are excluded from the edge list and handled as a local
    elementwise dinv^2 * xl term.
  - Each core computes y8 = fp8(64 * dinv * (x @ Wg.T)) for its 6400-slot
    slice into an SBUF staging tile (one DMA writes the whole slice),
    AllGather -> fp8 pair-row table [25600, 256] (pair = even/odd block
    at the same slot; int16 indices cover the whole table in ONE group).
  - Edge messages: dma_gather of 256B pair rows round-robined over the 4
    SWDGE queues (parallel descriptor generation -- the single biggest
    win: 8.5us -> 2.6us engine time per 1024-idx gather); per 128-edge
    chunk two one-hot matmuls (host-precomputed combined one-hot, parity
    lo/hi halves) accumulate the dst-block segment sum in PSUM.
  - h = relu(dinv/64 * S + dinv^2 * xl); v = colsum(h) + host-computed
    colsum(x)/8 via ones-matmul PSUM accumulation; AllReduce v; tiny MLP
    (weights prepped during phase B) replicated.
"""

import os
import numpy as np
import ml_dtypes

BF16 = ml_dtypes.bfloat16
F8 = ml_dtypes.float8_e4m3fn
N = 50000
E = 800000
D = 128
NCORES = 8
NB = 50             # dst blocks (bins) per core
NPAD = NB * 128     # 6400 padded node slots per core
NB2 = NB // 2       # even/odd block pairs
PAIRS = NPAD // 2   # 3200 pair rows per core
SEGC = int(os.environ.get("KB_SEGC", "8"))   # chunks per gather (8*128 = 1024 idx)
PADSLOT = 300.0
YSCALE = 64.0


def _pack_bins(deg):
    """Pack nodes into 8*49 bins (<=128 nodes, degree-sum target <=2048).

    Returns bins: list of node-index arrays, length NCORES*NB, ordered
    (core, block) with per-core blocks sorted by descending edge count.
    """
    B = NCORES * NB
    order = np.argsort(-deg, kind="stable")
    nbins = [[] for _ in range(B)]
    bsum = np.zeros(B, dtype=np.int64)
    # snake deal by descending degree
    pos = 0
    r = 0
    while pos < N:
        take = min(B, N - pos)
        idxs = order[pos:pos + take]
        tgt = range(take) if (r % 2 == 0) else range(take - 1, -1, -1)
        for k, b in enumerate(tgt):
            nbins[b].append(idxs[k])
            bsum[b] += deg[idxs[k]]
        pos += take
        r += 1
    # repair: move a node out of overfull bins into the emptiest bin with
    # spare node capacity, preferring degree close to the overflow
    cnt = np.array([len(x) for x in nbins])
    for _ in range(4000):
        o = int(np.argmax(bsum))
        if bsum[o] <= 2048:
            break
        need = bsum[o] - 2048
        # candidate bins: below target and with slot space
        cands = np.where((bsum < 2048) & (cnt < 128))[0]
        if len(cands) == 0:
            break
        u = int(cands[np.argmax(2048 - bsum[cands])])
        slack = 2048 - bsum[u]
        degs_o = np.array([deg[n] for n in nbins[o]])
        # move one node with degree in [need, slack] if possible
        pick = None
        ok = np.where((degs_o >= need) & (degs_o <= slack))[0]
        if len(ok):
            pick = int(ok[np.argmin(degs_o[ok])])
        else:
            ok = np.where(degs_o <= slack)[0]
            if len(ok):
                pick = int(ok[np.argmax(degs_o[ok])])
        if pick is None:
            break
        node = nbins[o].pop(pick)
        bsum[o] -= deg[node]
        cnt[o] -= 1
        nbins[u].append(node)
        bsum[u] += deg[node]
        cnt[u] += 1
    # drain the 32 globally-smallest bins toward <=1536 so the last 4
    # block ranks need only 12 chunks (keeps CH at ~784); correctness does
    # not depend on convergence (ct[] adapts)
    order_s = np.argsort(bsum)
    small = set(int(b) for b in order_s[:4 * NCORES])
    for _ in range(3):
        for b in list(small):
            guard = 0
            while bsum[b] > 1536 and guard < 400:
                guard += 1
                room = np.array([2048 - bsum[u] if (cnt[u] < 128 and
                                 u not in small) else -1
                                 for u in range(B)])
                u = int(np.argmax(room))
                if room[u] <= 0:
                    break
                degs_b = np.array([deg[n] for n in nbins[b]])
                want = min(int(room[u]), int(bsum[b] - 1536) + 16)
                fit = np.where(degs_b <= want)[0]
                if len(fit) == 0:
                    break
                j = int(fit[np.argmax(degs_b[fit])])
                dnode = nbins[b].pop(j)
                bsum[b] -= deg[dnode]
                cnt[b] -= 1
                nbins[u].append(dnode)
                bsum[u] += deg[dnode]
                cnt[u] += 1
    # deal bins to cores: snake by descending sum -> balanced cores, and
    # rank-aligned blocks (block r of every core has similar count)
    order_b = np.argsort(-bsum, kind="stable")
    core_bins = [[] for _ in range(NCORES)]
    for r in range(NB):
        seq = range(NCORES) if (r % 2 == 0) else range(NCORES - 1, -1, -1)
        for k, c in enumerate(seq):
            core_bins[c].append(order_b[r * NCORES + k])
    out = []
    for c in range(NCORES):
        bl = sorted(core_bins[c], key=lambda b: -bsum[b])
        out.extend(bl)
    return [np.array(nbins[b], dtype=np.int64) for b in out]


def _prep(edge_index):
    """Host-side sharding prep. Returns per-core data + uniform plan."""
    src = np.asarray(edge_index[0]).astype(np.int64)
    dst = np.asarray(edge_index[1]).astype(np.int64)

    deg = np.bincount(dst, minlength=N).astype(np.int64)      # in-degree
    dinv = (1.0 / np.sqrt(deg.astype(np.float64) + 1.0))       # with self-loop

    bins = _pack_bins(deg)

    # node -> (core, block, slot) and padded row id
    prow = np.zeros(N, dtype=np.int64)
    slot_of = np.zeros(N, dtype=np.int64)
    core_of = np.zeros(N, dtype=np.int64)
    blk_of = np.zeros(N, dtype=np.int64)
    for i, nodes in enumerate(bins):
        c, b = divmod(i, NB)
        prow[nodes] = c * NPAD + b * 128 + np.arange(len(nodes))
        slot_of[nodes] = np.arange(len(nodes))
        core_of[nodes] = c
        blk_of[nodes] = b

    # pair-row table mapping: local row q = b*128 + s  ->  pair
    # c*PAIRS + s*NB2 + b//2, parity b%2 (even/odd block pairs; matches the
    # [128 slot, NB2, 256] DRAM layout of y_slice)
    r_src = prow[src]
    c_src = r_src // NPAD
    q_src = r_src % NPAD
    b_src = q_src // 128
    s_src = q_src % 128
    pair = c_src * PAIRS + s_src * NB2 + (b_src // 2)
    parity = b_src % 2

    e_core = core_of[dst]
    e_blk = blk_of[dst]
    e_slot = slot_of[dst]

    # per-(core, block) counts -> uniform chunk plan
    cnt = np.zeros((NCORES, NB), dtype=np.int64)
    np.add.at(cnt, (e_core, e_blk), 1)
    ct = np.ceil(cnt.max(axis=0) / 128.0).astype(np.int64)     # [NB]
    CH = int(ct.sum())
    off = np.zeros(NB, dtype=np.int64)
    off[1:] = np.cumsum(ct)[:-1]

    in_extra = []
    for c in range(NCORES):
        m = e_core == c
        ep, es, ebl, epar = pair[m], e_slot[m], e_blk[m], parity[m]
        o = np.argsort(ebl, kind="stable")
        ep, es, ebl, epar = ep[o], es[o], ebl[o], epar[o]
        loc = np.zeros(CH * 128, dtype=np.int64)
        slo = np.full(CH * 128, PADSLOT, dtype=np.float64)
        shi = np.full(CH * 128, PADSLOT, dtype=np.float64)
        bstart = np.zeros(NB, dtype=np.int64)
        bstart[1:] = np.cumsum(np.bincount(ebl, minlength=NB))[:-1]
        pos = off[ebl] * 128 + (np.arange(len(ep)) - bstart[ebl])
        loc[pos] = ep
        slo[pos] = np.where(epar == 0, es, PADSLOT)
        shi[pos] = np.where(epar == 1, es, PADSLOT)

        # wrapped int16 index layout per segment
        nch = CH
        cols = []
        for s0 in range(0, nch, SEGC):
            seg = loc[s0 * 128: min(nch, s0 + SEGC) * 128]
            a = seg.reshape(-1, 16).T.astype(np.int16)
            cols.append(np.tile(a, (8, 1)))
        # host-built one-hot [128 edge-pos, CH, 256]: cols 0:128 = lo
        # parity, 128:256 = hi parity
        ohc = np.zeros((128, CH, 256), dtype=F8)
        ww = np.arange(CH * 128)
        epos = ww % 128
        echk = ww // 128
        lo_m = slo < 128
        hi_m = shi < 128
        ohc[epos[lo_m], echk[lo_m], slo[lo_m].astype(np.int64)] = 1.0
        ohc[epos[hi_m], echk[hi_m], 128 + shi[hi_m].astype(np.int64)] = 1.0
        in_extra.append({
            "idx": np.concatenate(cols, axis=1),
            "ohc": ohc.reshape(128, CH * 256),
        })

    plan = {"ct": ct, "CH": CH, "off": off}
    perm = {"prow": prow, "dinv": dinv, "bins": bins}
    return perm, in_extra, plan


def _build(plan, bias_info):
    import concourse.bacc as bacc
    import concourse.tile as tile
    from concourse import mybir

    f32 = mybir.dt.float32
    f16 = mybir.dt.bfloat16
    f8 = mybir.dt.float8e4
    i16 = mybir.dt.int16
    Alu = mybir.AluOpType
    Act = mybir.ActivationFunctionType

    ct, CH, off = plan["ct"], plan["CH"], plan["off"]
    has_bg, has_b1, has_b2, b3val = bias_info

    nc = bacc.Bacc("TRN2", target_bir_lowering=False, debug=False,
                   num_devices=NCORES, dynamic_dma_scratch_size=114688,
                   num_swdge_queues=4)

    def din(name, shape, dt=f32):
        return nc.dram_tensor(name, list(shape), dt, kind="ExternalInput")

    xs_d = din("xs", [NPAD, D], f16)   # pre-scaled by 64*dinv on host
    xsum_d = din("xsum", [1, 128])
    dvh_d = din("dvh", [128, NB])       # dinv / 64  (h + self scale)
    ones_d = din("onesf", [128, 1])
    idf_d = din("idf", [128, 128])
    idh_d = din("idh", [128, 128], f16)
    wg_d = din("wg", [D, D])
    w1_d = din("w1", [512, D])
    w2_d = din("w2", [256, 512])
    w3_d = din("w3", [1, 256])
    idx_d = din("idx", [128, CH * 8], i16)
    ohc_d = din("ohc", [128, CH * 256], f8)
    bgr_d = din("bgr", [1, 128]) if has_bg else None
    b1r_d = din("b1r", [1, 512]) if has_b1 else None
    b2r_d = din("b2r", [1, 256]) if has_b2 else None
    out_d = nc.dram_tensor("out", [1, 1], f32, kind="ExternalOutput")

    y_slice = nc.dram_tensor("y_slice", [128, NB2, 256], f8)
    y_full = nc.dram_tensor("y_full", [PAIRS * NCORES, 256], f8,
                            addr_space="Shared")
    vb = nc.dram_tensor("vb", [1, 128], f32)
    vr = nc.dram_tensor("vr", [1, 128], f32, addr_space="Shared")

    RG = [list(range(NCORES))]

    with tile.TileContext(nc) as tc:
        with (
            tc.tile_pool(name="const", bufs=1) as cpool,
            tc.tile_pool(name="xt", bufs=1) as xtpool,
            tc.tile_pool(name="xin", bufs=3) as xpool,
            tc.tile_pool(name="yt", bufs=3) as ypool,
            tc.tile_pool(name="seg", bufs=12) as segpool,
            tc.tile_pool(name="oh", bufs=12) as ohpool,
            tc.tile_pool(name="hb", bufs=3) as hpool,
            tc.tile_pool(name="mlp", bufs=1) as mpool,
            tc.tile_pool(name="psv", bufs=1, space="PSUM") as pvpool,
            tc.tile_pool(name="psS", bufs=5, space="PSUM") as pspool,
            tc.tile_pool(name="psT", bufs=2, space="PSUM") as ptpool,
        ):
            # ---- xT first: it gates the phase-B matmul chain (4 pieces
            # so early block groups unlock before the full transpose lands)
            xT = xtpool.tile([128, NPAD], f16)
            QR = NPAD // 4
            for q in range(4):
                nc.sync.dma_start(xT[:, q * QR:(q + 1) * QR],
                                  xs_d[q * QR:(q + 1) * QR, :],
                                  transpose=True)
            xsum_t = cpool.tile([1, 128], f32, tag="xsum")
            nc.scalar.dma_start(xsum_t[:], xsum_d[:])

            # ---- constants ----
            dvh_t = cpool.tile([128, NB], f32, tag="dvh")
            nc.sync.dma_start(dvh_t[:], dvh_d[:])
            ones_t = cpool.tile([128, 1], f32)
            nc.sync.dma_start(ones_t[:], ones_d[:])
            idf_t = cpool.tile([128, 128], f32)
            nc.sync.dma_start(idf_t[:], idf_d[:])
            idh_t = cpool.tile([128, 128], f16)
            nc.sync.dma_start(idh_t[:], idh_d[:])

            idx_t = cpool.tile([128, CH * 8], i16, tag="idx")
            nc.sync.dma_start(idx_t[:], idx_d[:])

            # Wg -> WgT fp16
            wg_t = cpool.tile([128, 128], f32, tag="wg32")
            nc.scalar.dma_start(wg_t[:], wg_d[:])
            wg_h = cpool.tile([128, 128], f16, tag="wg16")
            nc.vector.tensor_copy(wg_h[:], wg_t[:])
            ps = ptpool.tile([128, 128], f16, tag="pst")
            nc.tensor.transpose(ps[:], wg_h[:], idh_t[:])
            wgT = cpool.tile([128, 128], f16, tag="wgT")
            nc.vector.tensor_copy(wgT[:], ps[:])

            if has_bg:
                bgr_t = cpool.tile([1, 128], f32, tag="bgr")
                nc.sync.dma_start(bgr_t[:], bgr_d[:])
                psb = ptpool.tile([128, 128], f32, tag="pst")
                nc.tensor.matmul(psb[:], ones_t[:], bgr_t[:],
                                 start=True, stop=True)
                bg_t = cpool.tile([128, 128], f32, tag="bgt")
                nc.vector.tensor_copy(bg_t[:], psb[:])

            # ---- MLP weight prep (hoisted; independent of v) ----
            def transpose_to_sbuf(in_ap, p, f, dt=f32, tag=""):
                pst = ptpool.tile([f, p] if p > 1 else [f, 1], dt, tag="pst")
                ident = idf_t if dt == f32 else idh_t
                nc.tensor.transpose(pst[:], in_ap, ident[0:p, 0:p])
                sb = mpool.tile([f, p], dt, tag=tag)
                nc.vector.tensor_copy(sb[:], pst[:])
                return sb

            b1c = []
            if has_b1:
                b1r_t = mpool.tile([1, 512], f32, tag="b1r")
                nc.scalar.dma_start(b1r_t[:], b1r_d[:])
                for m in range(4):
                    b1c.append(transpose_to_sbuf(
                        b1r_t[:, m * 128:(m + 1) * 128], 1, 128,
                        tag=f"b1c{m}"))
            b2c = []
            if has_b2:
                b2r_t = mpool.tile([1, 256], f32, tag="b2r")
                nc.scalar.dma_start(b2r_t[:], b2r_d[:])
                for m in range(2):
                    b2c.append(transpose_to_sbuf(
                        b2r_t[:, m * 128:(m + 1) * 128], 1, 128,
                        tag=f"b2c{m}"))
            w1T_t = []
            for m in range(4):
                wt = mpool.tile([128, 128], f32, tag=f"w1blk{m}")
                nc.scalar.dma_start(wt[:], w1_d[m * 128:(m + 1) * 128, :])
                w1T_t.append(transpose_to_sbuf(wt[:], 128, 128,
                                               tag=f"w1T{m}"))
            w2T_t = []
            for m in range(2):
                row = []
                for kk in range(4):
                    wt = mpool.tile([128, 128], f32, tag=f"w2blk{m}{kk}")
                    nc.scalar.dma_start(
                        wt[:], w2_d[m * 128:(m + 1) * 128,
                                    kk * 128:(kk + 1) * 128])
                    row.append(transpose_to_sbuf(wt[:], 128, 128,
                                                 tag=f"w2T{m}{kk}"))
                w2T_t.append(row)
            w3r = mpool.tile([1, 256], f32, tag="w3r")
            nc.scalar.dma_start(w3r[:], w3_d[:])
            w3c_t = [transpose_to_sbuf(w3r[:, kk * 128:(kk + 1) * 128],
                                       1, 128, tag=f"w3c{kk}")
                     for kk in range(2)]

            # ---- phase B: y8 + self term ----
            psv = pvpool.tile([1, 128], f32)
            xself = xtpool.tile([128, NPAD], f16, tag="xself")
            first_mm = [True]

            def vacc(rhs_ap, stop=False):
                nc.tensor.matmul(psv[:], ones_t[:], rhs_ap,
                                 start=first_mm[0], stop=stop,
                                 skip_group_check=True)
                first_mm[0] = False

            ystage = xtpool.tile([128, NB2, 256], f8, tag="ystage")
            for g in range(0, NB, 4):
                nblk = min(4, NB - g)
                psy4 = pspool.tile([128, (nblk + 1) // 2, 256], f32,
                                   tag="psS")
                for k in range(nblk):
                    b = g + k
                    nc.tensor.matmul(psy4[:, k // 2,
                                          (k % 2) * 128:(k % 2) * 128 + 128],
                                     xT[:, b * 128:(b + 1) * 128], wgT[:],
                                     start=True, stop=True)
                    nc.vector.tensor_scalar(
                        xself[:, b * 128:(b + 1) * 128],
                        psy4[:, k // 2, (k % 2) * 128:(k % 2) * 128 + 128],
                        dvh_t[:, b:b + 1], None, Alu.mult)
                nc.scalar.activation(ystage[:, g // 2:g // 2 + (nblk + 1) // 2,
                                            :], psy4[:], Act.Copy)
            nc.sync.dma_start(y_slice[:], ystage[:])

            # ---- AllGather y (fp8 pair table) ----
            nc.gpsimd.collective_compute(
                "AllGather", mybir.AluOpType.bypass, replica_groups=RG,
                ins=[y_slice[:]], outs=[y_full[:]])

            # ---- main: gather + dual one-hot segment-sum ----
            nseg = int(np.ceil(CH / SEGC))
            seg_tiles = [None] * nseg
            oh_tiles = [None] * nseg
            seg_ptr = [0]

            def ensure_seg(s):
                while seg_ptr[0] <= s:
                    si = seg_ptr[0]
                    ncols = min(SEGC, CH - si * SEGC)
                    tl = segpool.tile([128, ncols, 256], f8, tag="seg")
                    nidx = ncols * 128
                    nc.gpsimd.dma_gather(
                        tl[:], y_full[:], idx_t[:, si * (SEGC * 8):
                                                si * (SEGC * 8) + ncols * 8],
                        num_idxs=nidx, num_idxs_reg=nidx,
                        elem_size=256, elem_step=256, single_packet=False,
                        queue_num=si % 4)
                    seg_tiles[si] = tl
                    c0 = si * SEGC * 256
                    ohc = ohpool.tile([128, ncols, 256], f8, tag="ohc")
                    nc.sync.dma_start(ohc[:], ohc_d[:, c0:c0 + ncols * 256])
                    oh_tiles[si] = ohc
                    seg_ptr[0] += 1

            for b in range(NB):
                psS = pspool.tile([128, 128], f32, tag="psS")
                tot = 2 * int(ct[b])
                k = 0
                for j in range(int(ct[b])):
                    ci = int(off[b]) + j
                    s, col = divmod(ci, SEGC)
                    ensure_seg(s)
                    ohc = oh_tiles[s]
                    nc.tensor.matmul(psS[:], ohc[:, col, 0:128],
                                     seg_tiles[s][:, col, 0:128],
                                     start=(k == 0), stop=False)
                    k += 1
                    nc.tensor.matmul(psS[:], ohc[:, col, 128:256],
                                     seg_tiles[s][:, col, 128:256],
                                     start=False, stop=(k == tot - 1))
                    k += 1
                hb = hpool.tile([128, 128], f32)
                tmp = hpool.tile([128, 128], f32, tag="tmp")
                nc.vector.tensor_scalar(tmp[:], psS[:], dvh_t[:, b:b + 1],
                                        None, Alu.mult)
                nc.vector.tensor_tensor(tmp[:], tmp[:],
                                        xself[:, b * 128:(b + 1) * 128],
                                        Alu.add)
                if has_bg:
                    nc.vector.tensor_tensor(tmp[:], tmp[:], bg_t[:], Alu.add)
                nc.scalar.activation(hb[:], tmp[:], Act.Relu)
                vacc(hb[:], stop=(b == NB - 1))

            # ---- v AllReduce ----
            vrow = mpool.tile([1, 128], f32, tag="vrow")
            nc.vector.tensor_tensor(vrow[:], psv[:], xsum_t[:], Alu.add)
            nc.sync.dma_start(vb[:], vrow[:])
            nc.gpsimd.collective_compute(
                "AllReduce", mybir.AluOpType.add, replica_groups=RG,
                ins=[vb[:]], outs=[vr[:]])
            vfull = mpool.tile([1, 128], f32, tag="vfull")
            nc.sync.dma_start(vfull[:], vr[:])

            # ---- MLP ----
            vcol = transpose_to_sbuf(vfull[:], 1, 128, tag="vcol")

            a1c = []
            for m in range(4):
                ps1 = ptpool.tile([128, 1], f32, tag="pst")
                nc.tensor.matmul(ps1[:], w1T_t[m][:], vcol[:], start=True,
                                 stop=True)
                a1 = mpool.tile([128, 1], f32, tag=f"a1c{m}")
                if has_b1:
                    nc.scalar.activation(a1[:], ps1[:], Act.Relu,
                                         bias=b1c[m][:])
                else:
                    nc.scalar.activation(a1[:], ps1[:], Act.Relu)
                a1c.append(a1)

            a2c = []
            for m in range(2):
                ps2 = ptpool.tile([128, 1], f32, tag="pst")
                for kk in range(4):
                    nc.tensor.matmul(ps2[:], w2T_t[m][kk][:], a1c[kk][:],
                                     start=(kk == 0), stop=(kk == 3))
                a2 = mpool.tile([128, 1], f32, tag=f"a2c{m}")
                if has_b2:
                    nc.scalar.activation(a2[:], ps2[:], Act.Relu,
                                         bias=b2c[m][:])
                else:
                    nc.scalar.activation(a2[:], ps2[:], Act.Relu)
                a2c.append(a2)

            ps3 = ptpool.tile([1, 1], f32, tag="pst")
            for kk in range(2):
                nc.tensor.matmul(ps3[:], w3c_t[kk][:], a2c[kk][:],
                                 start=(kk == 0), stop=(kk == 1))
            ot = mpool.tile([1, 1], f32, tag="ot")
            nc.scalar.activation(ot[:], ps3[:], Act.Copy, bias=float(b3val))
            nc.sync.dma_start(out_d[:], ot[:])

    nc.compile()
    return nc


TRACE = False
LAST_EXEC_NS = None
LAST_RESULT = None


def kernel(**inputs):
    from concourse.bass_utils import run_bass_kernel_spmd

    x = np.asarray(inputs["x"], dtype=np.float32)
    Wg = np.asarray(inputs["Wg"], dtype=np.float32)
    bg = np.asarray(inputs["bg"], dtype=np.float32)
    W1 = np.asarray(inputs["W1"], dtype=np.float32)
    b1 = np.asarray(inputs["b1"], dtype=np.float32)
    W2 = np.asarray(inputs["W2"], dtype=np.float32)
    b2 = np.asarray(inputs["b2"], dtype=np.float32)
    W3 = np.asarray(inputs["W3"], dtype=np.float32)
    b3 = np.asarray(inputs["b3"], dtype=np.float32)

    perm, in_extra, plan = _prep(inputs["edge_index"])
    bias_info = (bool(bg.any()), bool(b1.any()), bool(b2.any()),
                 float(b3.reshape(-1)[0]))
    nc = _build(plan, bias_info)

    idf = np.eye(128, dtype=np.float32)
    idh = np.eye(128).astype(BF16)
    ones = np.ones((128, 1), dtype=np.float32)

    prow, dinv = perm["prow"], perm["dinv"]

    def col_layout(vals_padded):
        return vals_padded.reshape(NB, 128).T.astype(np.float32).copy()

    xsum = (x.astype(np.float64).sum(axis=0) / NCORES).astype(np.float32)
    in_maps = []
    for c in range(NCORES):
        xs = np.zeros((NPAD, D), dtype=BF16)
        dv = np.zeros(NPAD, dtype=np.float64)
        mask = (prow >= c * NPAD) & (prow < (c + 1) * NPAD)
        nodes = np.where(mask)[0]
        local = prow[nodes] - c * NPAD
        xs[local] = (x[nodes].astype(np.float64)
                     * (YSCALE * dinv[nodes])[:, None]).astype(BF16)
        dv[local] = dinv[nodes]
        m = {"xs": xs, "xsum": xsum.reshape(1, 128),
             "dvh": col_layout(dv / YSCALE),
             "onesf": ones, "idf": idf, "idh": idh,
             "wg": Wg, "w1": W1, "w2": W2, "w3": W3.reshape(1, 256),
             "idx": in_extra[c]["idx"],
             "ohc": in_extra[c]["ohc"]}
        if bias_info[0]:
            m["bgr"] = bg.reshape(1, 128)
        if bias_info[1]:
            m["b1r"] = b1.reshape(1, 512)
        if bias_info[2]:
            m["b2r"] = b2.reshape(1, 256)
        in_maps.append(m)

    res = run_bass_kernel_spmd(nc, in_maps, list(range(NCORES)), trace=TRACE)
    global LAST_EXEC_NS, LAST_RESULT
    LAST_EXEC_NS = res.exec_time_ns
    LAST_RESULT = res
    return res.results[0]["out"].reshape(1).astype(np.float32)
